# revision 1
# baseline (speedup 1.0000x reference)
"""2-layer GCN forward on 8 TRN2 NeuronCores (Bass/bacc, raw engine streams).

Strategy:
  - Host filters the graph: output only needs rows idx -> layer-2 spmm only
    needs edges with dst in unique(idx) (~4% of edges); layer-1 spmm only
    needs h1 at the unique sources of those edges (n1 ~ 24k nodes).
  - Layer 1: dst-sharded across cores (12 windows of 256 dst-slots per core).
    Per 128-edge chunk: dma_gather x[src] rows (f32r), build scatter matrix
    S[e, d] = w_e * (d == dstc_e) on DVE/ACT, accumulate spmm^T = X^T S on PE
    into PSUM (f32r, moving dim 256 -> 1 cyc/row). Then h1 = relu(spmm@W1+b1).
    Sources are global node ids; int16 gather indices -> lo/hi base split.
  - Layer 2: src-sharded (each core owns its h1 slice; edges assigned to the
    src owner; gathers are core-local). Partial sums H2^T S2 = [hid, dst]
    accumulated per dst-window, AllReduce'd (1 MB), then @W2 + b2 +
    log_softmax computed (redundantly) on every core.
  - Host selects unique-idx rows and expands back via inverse permutation.
"""
import sys

for p in ("/opt/trn_rl_repo",):
    if p not in sys.path:
        sys.path.append(p)

import numpy as np

import concourse.bass as bass  # noqa: F401  (engine types)
import concourse.bacc as bacc
from concourse import library_config, mybir
from concourse.bass_utils import run_bass_kernel_spmd

F32 = mybir.dt.float32
F32R = mybir.dt.float32r
I16 = mybir.dt.int16
AL = mybir.AluOpType
AF = mybir.ActivationFunctionType
AX = mybir.AxisListType

NCORES = 8
NW1 = 12          # L1 dst windows per core
NW2 = 8           # L2 dst windows (global)
WIN = 256         # dst slots per window
M1 = NW1 * WIN    # 3072 h1 rows per core
N1P = NCORES * M1
N2P = NW2 * WIN   # 2048
HIB = 32768       # int16 index range split
ACT_FRAC = 0.25   # share of L1 S-builds done on ACT (batched, sem-ordered)

_PROG_CACHE = {}


def set_config(nw1=12, nw2=8, hib=32768, ncores=8):
    """Shrink the kernel for simulation/debug."""
    global NW1, NW2, M1, N1P, N2P, HIB, NCORES
    NW1, NW2, HIB, NCORES = nw1, nw2, hib, ncores
    M1 = NW1 * WIN
    N1P = NCORES * M1
    N2P = NW2 * WIN
    _PROG_CACHE.clear()


# ----------------------------------------------------------------- host prep
def _pack_idx16(stream):
    """[n] (n%16==0) -> [128, n//16] int16 in dma_gather layout."""
    a = stream.astype(np.int16).reshape(-1, 16).T  # [16, n/16]
    return np.tile(a, (8, 1))


def _chunk_major(arr_cw, K):
    """[nw, K*128] -> [128, nw*K] (chunk-major columns)."""
    nw = arr_cw.shape[0]
    return np.ascontiguousarray(
        arr_cw.reshape(nw * K, 128).T.astype(np.float32))


def _prep(x, W1, b1, W2, b2, edge_weight, src, dst, idx):
    n = x.shape[0]
    idx_u, idx_inv = np.unique(idx, return_inverse=True)
    n2 = len(idx_u)
    assert n2 <= N2P

    # L2 edges: dst in idx_u
    p2 = np.searchsorted(idx_u, dst).clip(0, n2 - 1)
    m2 = idx_u[p2] == dst
    e2s, e2w, d2 = src[m2], edge_weight[m2], p2[m2]

    S1 = np.unique(e2s)
    n1 = len(S1)
    assert n1 <= N1P, n1
    s2c = np.searchsorted(S1, e2s)

    # L1 edges: dst in S1
    p1 = np.searchsorted(S1, dst).clip(0, n1 - 1)
    m1 = S1[p1] == dst
    e1s, e1w, d1 = src[m1], edge_weight[m1], p1[m1]

    # ---- balance (lo, hi) edge counts across the (core, window) buckets:
    # permute compact node ids so no window needs extra padded chunks.
    nb = NCORES * NW1
    lo_deg = np.bincount(d1, weights=(e1s < HIB), minlength=n1)
    hi_deg = np.bincount(d1, weights=(e1s >= HIB), minlength=n1)
    tgt_lo = lo_deg.sum() / nb
    tgt_hi = hi_deg.sum() / nb
    order_n = np.argsort(-(lo_deg + hi_deg), kind="stable")
    bl = np.zeros(nb)
    bh = np.zeros(nb)
    bcnt = np.zeros(nb, np.int64)
    slot_of = np.empty(n1, np.int64)
    for node in order_n:
        cost = np.maximum((bl + lo_deg[node]) / max(tgt_lo, 1.0),
                          (bh + hi_deg[node]) / max(tgt_hi, 1.0))
        cost[bcnt >= WIN] = np.inf
        b = int(np.argmin(cost))
        slot_of[node] = b * WIN + bcnt[b]
        bl[b] += lo_deg[node]
        bh[b] += hi_deg[node]
        bcnt[b] += 1
    # perm: compact id -> balanced position (unused tail positions stay free)
    d1 = slot_of[d1]
    s2c = slot_of[s2c]

    # ---- L1 packing: (core, window, lo/hi) buckets
    core1 = d1 // M1
    win1 = (d1 % M1) // WIN
    slot1 = d1 % WIN
    hi1 = (e1s >= HIB).astype(np.int64)
    key1 = (core1 * NW1 + win1) * 2 + hi1
    nk1 = NCORES * NW1 * 2
    cnt1 = np.bincount(key1, minlength=nk1).reshape(NCORES, NW1, 2)
    K1LO = int(-(-cnt1[:, :, 0].max() // 128))
    K1HI = int(-(-cnt1[:, :, 1].max() // 128))
    K1 = K1LO + K1HI

    order = np.argsort(key1, kind="stable")
    ks = key1[order]
    starts = np.searchsorted(ks, np.arange(nk1))
    rank = np.arange(len(ks)) - starts[ks]
    base = np.where(ks % 2 == 0, 0, K1LO * 128)
    pos = rank + base  # slot within window stream

    idx1 = np.zeros((NCORES, NW1, K1 * 128), np.int32)
    w1s = np.zeros((NCORES, NW1, K1 * 128), np.float32)
    d1s = np.zeros((NCORES, NW1, K1 * 128), np.float32)
    oc = core1[order]
    ow = win1[order]
    sv = e1s[order] - np.where(ks % 2 == 0, 0, HIB)
    idx1[oc, ow, pos] = sv
    w1s[oc, ow, pos] = e1w[order]
    d1s[oc, ow, pos] = slot1[order]

    # ---- L2 packing: (owner-core-by-src, window) buckets
    core2 = s2c // M1
    loc2 = s2c % M1
    win2 = d2 // WIN
    slot2 = d2 % WIN
    key2 = core2 * NW2 + win2
    nk2 = NCORES * NW2
    cnt2 = np.bincount(key2, minlength=nk2).reshape(NCORES, NW2)
    K2 = int(-(-cnt2.max() // 128))

    order = np.argsort(key2, kind="stable")
    ks = key2[order]
    starts = np.searchsorted(ks, np.arange(nk2))
    rank = np.arange(len(ks)) - starts[ks]
    idx2 = np.zeros((NCORES, NW2, K2 * 128), np.int32)
    w2s = np.zeros((NCORES, NW2, K2 * 128), np.float32)
    d2s = np.zeros((NCORES, NW2, K2 * 128), np.float32)
    oc = core2[order]
    ow = win2[order]
    idx2[oc, ow, rank] = loc2[order]
    w2s[oc, ow, rank] = e2w[order]
    d2s[oc, ow, rank] = slot2[order]

    # ---- meta layout (shared columns for all cores; stream parts per-core)
    iota = np.broadcast_to(np.arange(WIN, dtype=np.float32), (128, WIN))
    b1b = np.broadcast_to(b1.astype(np.float32), (128, 128))
    b2b = np.broadcast_to(b2.astype(np.float32), (128, 40))
    W1a = W1[0:128, :].astype(np.float32)
    W1b = W1[128:256, :].astype(np.float32)
    W2c = W2.astype(np.float32)  # [128, 40]

    in_maps = []
    for c in range(NCORES):
        m = [iota,
             _chunk_major(w1s[c], K1), _chunk_major(d1s[c], K1),
             -_chunk_major(w1s[c], K1), -_chunk_major(d1s[c], K1),
             _chunk_major(w2s[c], K2), _chunk_major(d2s[c], K2),
             W1a, W1b, W2c, b1b, b2b]
        meta = np.ascontiguousarray(np.concatenate(m, axis=1))
        in_maps.append({
            "x": np.ascontiguousarray(x, dtype=np.float32),
            "idx1lo": _pack_idx16(
                idx1[c, :, :K1LO * 128].reshape(-1)),
            "idx1hi": _pack_idx16(
                idx1[c, :, K1LO * 128:].reshape(-1)),
            "idx2": _pack_idx16(idx2[c].reshape(-1)),
            "meta": meta,
        })
    return in_maps, (K1LO, K1HI, K2), idx_u, idx_inv, n2


# --------------------------------------------------------------- the program
def _meta_cols(K1, K2):
    """Column offsets within meta."""
    o = {}
    cur = 0
    for name, width in [("iota", WIN),
                        ("w1", NW1 * K1), ("d1", NW1 * K1),
                        ("nw1", NW1 * K1), ("nd1", NW1 * K1),
                        ("w2", NW2 * K2), ("d2", NW2 * K2),
                        ("W1a", 128), ("W1b", 128), ("W2", 40),
                        ("b1b", 128), ("b2b", 40)]:
        o[name] = cur
        cur += width
    o["total"] = cur
    return o


def _build(n_nodes, K1LO, K1HI, K2, reps=1):
    K1 = K1LO + K1HI
    MC = _meta_cols(K1, K2)
    nc = bacc.Bacc("TRN2", target_bir_lowering=False, debug=False,
                   num_devices=NCORES)

    x_t = nc.dram_tensor("x", [n_nodes, 256], F32R, kind="ExternalInput")
    i1lo_t = nc.dram_tensor("idx1lo", [128, NW1 * K1LO * 8], I16,
                            kind="ExternalInput")
    i1hi_t = nc.dram_tensor("idx1hi", [128, NW1 * K1HI * 8], I16,
                            kind="ExternalInput")
    i2_t = nc.dram_tensor("idx2", [128, NW2 * K2 * 8], I16,
                          kind="ExternalInput")
    meta_t = nc.dram_tensor("meta", [128, MC["total"]], F32,
                            kind="ExternalInput")
    out_t = nc.dram_tensor("out", [N2P, 40], F32, kind="ExternalOutput")
    h1_d = nc.dram_tensor("h1_d", [M1, 128], F32R)
    ar_in = nc.dram_tensor("ar_in", [128, N2P], F32)
    ar_out = nc.dram_tensor("ar_out", [128, N2P], F32, addr_space="Shared")

    import contextlib
    ctx = contextlib.ExitStack()
    sb = lambda name, shape, dt: ctx.enter_context(
        nc.sbuf_tensor(name, shape, dt))
    ps = lambda name, shape: ctx.enter_context(
        nc.psum_tensor(name, shape, F32))
    sem = lambda name: ctx.enter_context(nc.semaphore(name))

    i1lo = sb("i1lo", [128, NW1 * K1LO * 8], I16)
    i1hi = sb("i1hi", [128, NW1 * K1HI * 8], I16)
    i2 = sb("i2", [128, NW2 * K2 * 8], I16)
    meta = sb("meta_sb", [128, MC["total"]], F32)
    H1 = sb("H1", [128, 2, K1, 256], F32R)
    Srng = sb("Srng", [128, 2 * K1, WIN], F32R)
    H2 = sb("H2", [128, 2, K2, 128], F32R)
    spT = sb("spT", [128, 2, 2, 256], F32)
    h1sb = sb("h1sb", [128, 2, 2, 128], F32R)
    l2p = sb("l2p", [128, N2P], F32)
    arsb = sb("arsb", [128, N2P], F32)
    lssb = sb("lssb", [128, 2, 40], F32)
    escr = sb("escr", [128, 2, 40], F32)
    red = sb("red", [128, 64], F32)
    uscr = sb("uscr", [128, 2, 4, WIN], F32)

    # one full 2KB bank per tensor: avoids cross-tensor same-bank
    # PE-write/DVE-read hazards
    psA = [ps("psA0", [128, 512]), ps("psA1", [128, 512])]
    psB = [ps("psB0", [128, 512]), ps("psB1", [128, 512])]
    psH = [ps("psH0", [128, 512]), ps("psH1", [128, 512])]

    hw_s = sem("hw_s")
    g_r = [sem(f"g_r{i}") for i in range(4)]
    h1_r = [sem(f"h1_r{i}") for i in range(2)]
    out_r = [sem(f"out_r{i}") for i in range(2)]
    s_s = sem("s_s")
    pe_s = sem("pe_s")
    v_s = sem("v_s")
    a_s = sem("a_s")
    c_s = sem("c_s")

    # ---- schedule builder: append closures per engine with exact sem counts
    SP, PL, PE, DVE, ACT = "sp", "pl", "pe", "dve", "act"
    sched = {e: [] for e in (SP, PL, PE, DVE, ACT)}
    cnt = dict(hw=0, s=0, pe=0, v=0, a=0, c=0,
               g0=0, g1=0, g2=0, g3=0, h0=0, h1=0, o0=0, o1=0)
    semmap = dict(hw=hw_s, s=s_s, pe=pe_s, v=v_s, a=a_s, c=c_s,
                  g0=g_r[0], g1=g_r[1], g2=g_r[2], g3=g_r[3],
                  h0=h1_r[0], h1=h1_r[1], o0=out_r[0], o1=out_r[1])

    def wait(e, semname, val):
        if val <= 0:
            return
        s_ = semmap[semname]
        sched[e].append(lambda eng: eng.wait_ge(s_, val))

    def bump(semname, n=1):
        cnt[semname] += n
        return cnt[semname]

    def op(e, fn, semname=None, n=1):
        """fn(eng) must return the instruction; attach then_inc."""
        if semname is None:
            sched[e].append(fn)
            return None
        s_ = semmap[semname]
        v = bump(semname, n)
        sched[e].append(lambda eng: fn(eng).then_inc(s_, n))
        return v

    n_dve1 = K1 - int(round(K1 * ACT_FRAC))  # chunks/window on DVE

    # --- preamble
    op(SP, lambda e: e.dma_start(out=i1lo[:], in_=i1lo_t.ap()), "hw", 16)
    op(SP, lambda e: e.dma_start(out=i1hi[:], in_=i1hi_t.ap()), "hw", 16)
    op(SP, lambda e: e.dma_start(out=i2[:], in_=i2_t.ap()), "hw", 16)
    hw_pre = op(SP, lambda e: e.dma_start(out=meta[:], in_=meta_t.ap()),
                "hw", 16)
    sched[PL].append(lambda eng: eng.load_library(library_config.mlp))
    regs = {}
    GMAX = 8  # max chunks per dma_gather (>1024 idxs per instr breaks)

    def pieces(K):
        out = []
        c0 = 0
        while c0 < K:
            c1 = min(c0 + GMAX, K)
            out.append((c0, c1))
            c0 = c1
        return out

    def plreg(val):
        def fn(eng):
            regs[val] = eng.to_reg(val)
        sched[PL].append(fn)
    _sizes = set()
    for K in (K1LO, K1HI, K2):
        for c0, c1 in pieces(K):
            _sizes.add((c1 - c0) * 128)
    for v in sorted(_sizes):
        plreg(v)
    wait(PL, "hw", hw_pre)
    wait(DVE, "hw", hw_pre)
    wait(ACT, "hw", hw_pre)
    wait(PE, "hw", hw_pre)

    pe_chunks = {}
    pe_w1 = {}
    copyv = {}
    biasv = {}
    reluv = {}
    h1hw = {}
    gv1 = {}
    sv1 = {}

    def mcol(name, k=0, width=1):
        c0 = MC[name] + k * width
        return meta[:, c0:c0 + width]

    def s_build(e, slot, ccol, layer):
        """one S chunk build on DVE. ccol = stream column index."""
        w_ap = mcol("w1" if layer == 1 else "w2", ccol)
        d_ap = mcol("d1" if layer == 1 else "d2", ccol)
        Sap = Srng[:, slot, :]
        op(DVE, lambda eng: eng.tensor_scalar(
            Sap, meta[:, 0:WIN], d_ap, w_ap, AL.is_equal, AL.mult),
            "s")

    act_grp = {"i": 0, "prev_relu": {}}

    def s_build_act_group(slots_ccols):
        """<=4 S chunks on ACT: Abs batch -> sem -> Relu batch -> sem.
        ACT has no auto-drain, so RAW/WAR need explicit sem round-trips."""
        gi = act_grp["i"]
        bank = gi % 2
        # WAR: this group's Abs overwrites the bank read by Relus of group
        # gi-2; those Relus inc a_s, so a value-based wait orders us.
        if gi - 2 in act_grp["prev_relu"]:
            wait(ACT, "a", act_grp["prev_relu"][gi - 2])
        last = None
        for i, (slot, ccol) in enumerate(slots_ccols):
            nd_ap = mcol("nd1", ccol)
            last = op(ACT, lambda eng, i=i, nd_ap=nd_ap: eng.activation(
                uscr[:, bank, i, :], meta[:, 0:WIN], AF.Abs, bias=nd_ap),
                "a")
        wait(ACT, "a", last)
        rl = None
        for i, (slot, ccol) in enumerate(slots_ccols):
            w_ap = mcol("w1", ccol)
            nw_ap = mcol("nw1", ccol)
            rl = op(ACT, lambda eng, i=i, slot=slot, w_ap=w_ap, nw_ap=nw_ap:
                    eng.activation(
                Srng[:, slot, :], uscr[:, bank, i, :], AF.Relu,
                bias=w_ap, scale=nw_ap), "a")
        act_grp["prev_relu"][gi] = rl
        act_grp["i"] = gi + 1
        return rl

    def rep_barrier():
        # full cross-engine barrier between repetitions
        engines = (SP, PL, PE, DVE, ACT)
        names = ("pe", "v", "a", "s", "c", "hw",
                 "g0", "g1", "g2", "g3", "h0", "h1", "o0", "o1")
        for e in engines:
            for nm in names:
                wait(e, nm, cnt[nm])

    def emit_pipeline():
        pe_chunks = {}
        pe_w1 = {}
        copyv = {}
        biasv = {}
        reluv = {}
        h1hw = {}
        gv1 = {}
        sv1 = {}

        # --- L1 pipeline
        for w in range(NW1 + 1):
            if w < NW1:
                r = w % 2
                if w >= 2:
                    wait(PL, "pe", pe_chunks[w - 2])
                for c0, c1 in pieces(K1LO):
                    n_ = (c1 - c0) * 128
                    op(PL, lambda e, r=r, w=w, c0=c0, c1=c1, n_=n_:
                       e.dma_gather(
                        H1[:, r, c0:c1, :], x_t.ap(),
                        i1lo[:, (w * K1LO + c0) * 8:(w * K1LO + c1) * 8],
                        n_, regs[n_], 256), f"g{w % 4}", 16)
                for c0, c1 in pieces(K1HI):
                    n_ = (c1 - c0) * 128
                    gv1[w] = op(PL, lambda e, r=r, w=w, c0=c0, c1=c1, n_=n_:
                                e.dma_gather(
                        H1[:, r, K1LO + c0:K1LO + c1, :], x_t.ap()[HIB:, :],
                        i1hi[:, (w * K1HI + c0) * 8:(w * K1HI + c1) * 8],
                        n_, regs[n_], 256), f"g{w % 4}", 16)
                if w >= 2:
                    wait(DVE, "pe", pe_chunks[w - 2])
                    wait(ACT, "pe", pe_chunks[w - 2])
                for c in range(n_dve1):
                    s_build(DVE, r * K1 + c, w * K1 + c, 1)
                acts = [(r * K1 + c, w * K1 + c)
                        for c in range(n_dve1, K1)]
                av1 = None
                for j in range(0, len(acts), 4):
                    av1 = s_build_act_group(acts[j:j + 4])
                sv1[w] = cnt["s"]
                wait(PE, f"g{w % 4}", gv1[w])
                wait(PE, "s", sv1[w])
                if av1 is not None:
                    wait(PE, "a", av1)
                if w >= 2:
                    wait(PE, "v", copyv[w - 2])
                for c in range(K1):
                    st, sp_ = (c == 0), (c == K1 - 1)
                    op(PE, lambda e, r=r, c=c, st=st, sp_=sp_: e.matmul(
                        psA[r][:, 0:256], H1[:, r, c, 0:128],
                        Srng[:, r * K1 + c, :],
                        start=st, stop=sp_, skip_group_check=True))
                    last = op(PE, lambda e, r=r, c=c, st=st, sp_=sp_:
                              e.matmul(
                        psB[r][:, 0:256], H1[:, r, c, 128:256],
                        Srng[:, r * K1 + c, :],
                        start=st, stop=sp_, skip_group_check=True),
                        "pe" if sp_ else None)
                pe_chunks[w] = last
            if w >= 1:
                wa = w - 1
                ra = wa % 2
                wait(DVE, "pe", pe_chunks[wa])
                op(DVE, lambda e, ra=ra: e.tensor_copy(
                    spT[:, ra, 0, :], psA[ra][:, 0:256]), "v")
                copyv[wa] = op(DVE, lambda e, ra=ra: e.tensor_copy(
                    spT[:, ra, 1, :], psB[ra][:, 0:256]), "v")
                wait(PE, "v", copyv[wa])
                if wa >= 2:
                    wait(PE, "a", reluv[wa - 2])
                for dh in range(2):
                    for k in range(2):
                        pe_w1[wa] = op(PE, lambda e, ra=ra, dh=dh, k=k:
                                       e.matmul(
                            psH[ra][:, dh * 128:(dh + 1) * 128],
                            spT[:, ra, k, dh * 128:(dh + 1) * 128],
                            mcol("W1a" if k == 0 else "W1b", 0, 128),
                            start=(k == 0), stop=(k == 1),
                            skip_group_check=True),
                            "pe" if (dh == 1 and k == 1) else None)
                wait(DVE, "pe", pe_w1[wa])
                for dh in range(2):
                    biasv[wa] = op(DVE, lambda e, ra=ra, dh=dh:
                                   e.tensor_tensor(
                        psH[ra][:, dh * 128:(dh + 1) * 128],
                        psH[ra][:, dh * 128:(dh + 1) * 128],
                        mcol("b1b", 0, 128), AL.add), "v")
                wait(ACT, "v", biasv[wa])
                if wa >= 2:
                    wait(ACT, f"h{wa % 2}", h1hw[wa - 2])
                for dh in range(2):
                    reluv[wa] = op(ACT, lambda e, ra=ra, dh=dh: e.activation(
                        h1sb[:, ra, dh, :],
                        psH[ra][:, dh * 128:(dh + 1) * 128],
                        AF.Relu), "a")
                wait(SP, "a", reluv[wa])
                for dh in range(2):
                    h1hw[wa] = op(SP, lambda e, ra=ra, wa=wa, dh=dh:
                                  e.dma_start(
                        out=h1_d.ap()[wa * 256 + dh * 128:
                                      wa * 256 + (dh + 1) * 128, :],
                        in_=h1sb[:, ra, dh, :]), f"h{wa % 2}", 16)

        # --- L2 pipeline
        wait(PL, "h0", cnt["h0"])
        wait(PL, "h1", cnt["h1"])
        wait(DVE, "pe", pe_chunks[NW1 - 1])
        wait(ACT, "pe", pe_chunks[NW1 - 1])
        pe_l2 = {}
        gv2 = {}
        sv2 = {}
        cl2 = {}
        for v2 in range(NW2 + 1):
            if v2 < NW2:
                r = v2 % 2
                if v2 >= 2:
                    wait(PL, "pe", pe_l2[v2 - 2])
                for c0, c1 in pieces(K2):
                    n_ = (c1 - c0) * 128
                    gv2[v2] = op(PL, lambda e, r=r, v2=v2, c0=c0, c1=c1,
                                 n_=n_: e.dma_gather(
                        H2[:, r, c0:c1, :], h1_d.ap(),
                        i2[:, (v2 * K2 + c0) * 8:(v2 * K2 + c1) * 8],
                        n_, regs[n_], 128), f"g{v2 % 4}", 16)
                if v2 >= 2:
                    wait(DVE, "pe", pe_l2[v2 - 2])
                for c in range(K2):
                    s_build(DVE, r * K1 + c, v2 * K2 + c, 2)
                sv2[v2] = cnt["s"]
                wait(PE, f"g{v2 % 4}", gv2[v2])
                wait(PE, "s", sv2[v2])
                wait(PE, "v", copyv[NW1 - 2 + r] if v2 < 2 else cl2[v2 - 2])
                for c in range(K2):
                    st, sp_ = (c == 0), (c == K2 - 1)
                    pe_l2[v2] = op(PE, lambda e, r=r, c=c, st=st, sp_=sp_:
                                   e.matmul(
                        psA[r][:, 0:256], H2[:, r, c, :],
                        Srng[:, r * K1 + c, :],
                        start=st, stop=sp_, skip_group_check=True),
                        "pe" if sp_ else None)
            if v2 >= 1:
                va = v2 - 1
                ra = va % 2
                wait(DVE, "pe", pe_l2[va])
                cl2[va] = op(DVE, lambda e, ra=ra, va=va: e.tensor_copy(
                    l2p[:, va * 256:(va + 1) * 256], psA[ra][:, 0:256]), "v")

        # --- AllReduce
        wait(SP, "v", cl2[NW2 - 1])
        hw_l2p = op(SP, lambda e: e.dma_start(out=ar_in.ap(), in_=l2p[:]),
                    "hw", 16)
        wait(PL, "hw", hw_l2p)
        op(PL, lambda e: e.collective_compute(
            "AllReduce", AL.add, ins=[ar_in.ap()], outs=[ar_out.ap()],
            replica_groups=[list(range(NCORES))]), "c")
        wait(SP, "c", cnt["c"])
        hw_ar = op(SP, lambda e: e.dma_start(out=arsb[:], in_=ar_out.ap()),
                   "hw", 16)

        # --- final: W2 matmul + bias + log_softmax per (window, dhalf)
        wait(PE, "hw", hw_ar)
        wait(PE, "v", copyv[NW1 - 1])
        fin_a = {}
        out_hw = {}
        for g in range(2 * NW2):
            vw, dh = g // 2, g % 2
            pf = psB[g % 2]
            if g >= 2:
                wait(PE, "a", fin_a[g - 2])
            pev = op(PE, lambda e, vw=vw, dh=dh, pf=pf: e.matmul(
                pf[:, 0:40], arsb[:, vw * 256 + dh * 128:
                                  vw * 256 + (dh + 1) * 128],
                mcol("W2", 0, 40), start=True, stop=True,
                skip_group_check=True), "pe")
            wait(DVE, "pe", pev)
            rc = (g % 2) * 8
            bav = op(DVE, lambda e, pf=pf: e.tensor_tensor(
                pf[:, 0:40], pf[:, 0:40], mcol("b2b", 0, 40), AL.add), "v")
            wait(DVE, "v", bav)
            nmv = op(DVE, lambda e, pf=pf, rc=rc: e.tensor_reduce(
                red[:, rc:rc + 1], pf[:, 0:40], AX.X, AL.max,
                negate=True), "v")
            wait(ACT, "v", nmv)
            if g >= 2:
                wait(ACT, f"o{g % 2}", out_hw[g - 2])
            ev = op(ACT, lambda e, pf=pf, rc=rc, dh=dh: e.activation(
                escr[:, dh, :], pf[:, 0:40], AF.Exp, bias=red[:, rc:rc + 1],
                accum_out=red[:, rc + 1:rc + 2]), "a")
            wait(DVE, "a", ev)
            rv = op(DVE, lambda e, rc=rc: e.reciprocal(
                red[:, rc + 2:rc + 3], red[:, rc + 1:rc + 2]), "v")
            wait(ACT, "v", rv)
            lv = op(ACT, lambda e, rc=rc: e.activation(
                red[:, rc + 3:rc + 4], red[:, rc + 2:rc + 3], AF.Ln), "a")
            wait(DVE, "a", lv)
            mv = op(DVE, lambda e, rc=rc: e.tensor_tensor(
                red[:, rc + 4:rc + 5], red[:, rc:rc + 1],
                red[:, rc + 3:rc + 4], AL.add), "v")
            wait(ACT, "v", mv)
            fin_a[g] = op(ACT, lambda e, pf=pf, rc=rc, dh=dh: e.activation(
                lssb[:, dh, :], pf[:, 0:40], AF.Identity,
                bias=red[:, rc + 4:rc + 5]), "a")
            wait(SP, "a", fin_a[g])
            out_hw[g] = op(SP, lambda e, vw=vw, dh=dh: e.dma_start(
                out=out_t.ap()[vw * 256 + dh * 128:
                               vw * 256 + (dh + 1) * 128, :],
                in_=lssb[:, dh, :]), f"o{g % 2}", 16)

    emit_pipeline()
    for _ in range(reps - 1):
        rep_barrier()
        emit_pipeline()

    # ---- emit engine bodies
    with nc.Block() as block:
        @block.sync
        def _(eng):
            for fn in sched[SP]:
                fn(eng)

        @block.gpsimd
        def _(eng):
            for fn in sched[PL]:
                fn(eng)

        @block.tensor
        def _(eng):
            for fn in sched[PE]:
                fn(eng)

        @block.vector
        def _(eng):
            for fn in sched[DVE]:
                fn(eng)

        @block.scalar
        def _(eng):
            for fn in sched[ACT]:
                fn(eng)

    ctx.close()
    nc.compile()
    return nc


def get_program(n_nodes, K1LO, K1HI, K2, reps=1):
    key = (n_nodes, K1LO, K1HI, K2, reps)
    if key not in _PROG_CACHE:
        _PROG_CACHE[key] = _build(n_nodes, K1LO, K1HI, K2, reps)
    return _PROG_CACHE[key]


# ------------------------------------------------------------------- kernel
def kernel(x, W1, b1, W2, b2, edge_weight, src, dst, idx, _trace=False):
    x = np.asarray(x, np.float32)
    in_maps, (K1LO, K1HI, K2), idx_u, idx_inv, n2 = _prep(
        x, np.asarray(W1, np.float32), np.asarray(b1, np.float32),
        np.asarray(W2, np.float32), np.asarray(b2, np.float32),
        np.asarray(edge_weight, np.float32),
        np.asarray(src, np.int32), np.asarray(dst, np.int32),
        np.asarray(idx, np.int32))
    nc = get_program(x.shape[0], K1LO, K1HI, K2)
    res = run_bass_kernel_spmd(nc, in_maps, core_ids=list(range(NCORES)),
                               trace=_trace)
    full = res.results[0]["out"]  # [N2P, 40], rows 0..n2-1 = unique idx
    out = full[idx_inv].astype(np.float32)
    if _trace:
        kernel.last_exec_time_ns = res.exec_time_ns
    return out


kernel.last_exec_time_ns = None



# revision 3
# speedup vs baseline: 3.0519x; 3.0519x over previous
"""2-layer GCN forward on 8 TRN2 NeuronCores (Bass/bacc, raw engine streams).

Strategy:
  - Host filters the graph: output only needs rows idx -> layer-2 spmm only
    needs edges with dst in unique(idx) (~4% of edges); layer-1 spmm only
    needs h1 at the unique sources of those edges (n1 ~ 24k nodes).
  - Layer 1: dst-sharded across cores (12 windows of 256 dst-slots per core).
    Per 128-edge chunk: dma_gather x[src] rows (bf16, 512B rows), build
    scatter matrix S[e, d] = w_e * (d == dstc_e) in bf16 on DVE/ACT,
    accumulate spmm^T = X^T S on PE into PSUM. Then h1 = relu(spmm@W1+b1)
    (W1 in bf16, spT copy in bf16 for 1 cyc/row moving).
  - Layer 2: src-sharded (each core owns its h1 slice, stored bf16; edges
    assigned to the src owner; gathers are core-local). Matmul orientation
    flipped vs layer 1: partial sums S2^T H2 = [dst, hid] per window, so the
    per-core partial tensor is [2048 dst, 128 hid] laid out dst-major.
  - ReduceScatter (instead of AllReduce) along the dst axis: core c receives
    exactly its two 128-row dst blocks. Final @W2 + b2 + log_softmax is
    per-core on its own 256 rows (PE-transpose, bf16 W2, Exp/Ln batched so
    only ~2 activation-table loads happen per iteration).
  - Host reassembles the 8 per-core [256, 40] outputs and expands to idx
    via the inverse permutation.
"""
import sys

for p in ("/opt/trn_rl_repo",):
    if p not in sys.path:
        sys.path.append(p)

import numpy as np

import concourse.bass as bass  # noqa: F401  (engine types)
import concourse.bacc as bacc
from concourse import library_config, mybir
from concourse.bass_utils import run_bass_kernel_spmd

F32 = mybir.dt.float32
F32R = mybir.dt.float32r
BF16 = mybir.dt.bfloat16
I16 = mybir.dt.int16
AL = mybir.AluOpType
AF = mybir.ActivationFunctionType
AX = mybir.AxisListType
NPBF16 = mybir.dt.np(BF16)

NCORES = 8
NW1 = 12          # L1 dst windows per core
NW2 = 8           # L2 dst windows (global)
WIN = 256         # dst slots per window
M1 = NW1 * WIN    # 3072 h1 rows per core
N1P = NCORES * M1
N2P = NW2 * WIN   # 2048
HIB = 32768       # int16 index range split
ACT_FRAC = 0.25   # share of L1 S-builds done on ACT (batched, sem-ordered)

_PROG_CACHE = {}


def set_config(nw1=12, nw2=8, hib=32768, ncores=8):
    """Shrink the kernel for simulation/debug."""
    global NW1, NW2, M1, N1P, N2P, HIB, NCORES
    NW1, NW2, HIB, NCORES = nw1, nw2, hib, ncores
    M1 = NW1 * WIN
    N1P = NCORES * M1
    N2P = NW2 * WIN
    _PROG_CACHE.clear()


# ----------------------------------------------------------------- host prep
def _pack_idx16(stream):
    """[n] (n%16==0) -> [128, n//16] int16 in dma_gather layout."""
    a = stream.astype(np.int16).reshape(-1, 16).T  # [16, n/16]
    return np.tile(a, (8, 1))


def _chunk_major(arr_cw, K):
    """[nw, K*128] -> [128, nw*K] (chunk-major columns)."""
    nw = arr_cw.shape[0]
    return np.ascontiguousarray(
        arr_cw.reshape(nw * K, 128).T.astype(np.float32))


def _prep(x, W1, b1, W2, b2, edge_weight, src, dst, idx):
    n = x.shape[0]
    idx_u, idx_inv = np.unique(idx, return_inverse=True)
    n2 = len(idx_u)
    assert n2 <= N2P

    # L2 edges: dst in idx_u
    p2 = np.searchsorted(idx_u, dst).clip(0, n2 - 1)
    m2 = idx_u[p2] == dst
    e2s, e2w, d2 = src[m2], edge_weight[m2], p2[m2]

    S1 = np.unique(e2s)
    n1 = len(S1)
    assert n1 <= N1P, n1
    s2c = np.searchsorted(S1, e2s)

    # L1 edges: dst in S1
    p1 = np.searchsorted(S1, dst).clip(0, n1 - 1)
    m1 = S1[p1] == dst
    e1s, e1w, d1 = src[m1], edge_weight[m1], p1[m1]

    # ---- balance (lo, hi) edge counts across the (core, window) buckets:
    # permute compact node ids so no window needs extra padded chunks.
    nb = NCORES * NW1
    lo_deg = np.bincount(d1, weights=(e1s < HIB), minlength=n1)
    hi_deg = np.bincount(d1, weights=(e1s >= HIB), minlength=n1)
    tgt_lo = lo_deg.sum() / nb
    tgt_hi = hi_deg.sum() / nb
    order_n = np.argsort(-(lo_deg + hi_deg), kind="stable")
    bl = np.zeros(nb)
    bh = np.zeros(nb)
    bcnt = np.zeros(nb, np.int64)
    slot_of = np.empty(n1, np.int64)
    for node in order_n:
        cost = np.maximum((bl + lo_deg[node]) / max(tgt_lo, 1.0),
                          (bh + hi_deg[node]) / max(tgt_hi, 1.0))
        cost[bcnt >= WIN] = np.inf
        b = int(np.argmin(cost))
        slot_of[node] = b * WIN + bcnt[b]
        bl[b] += lo_deg[node]
        bh[b] += hi_deg[node]
        bcnt[b] += 1
    # perm: compact id -> balanced position (unused tail positions stay free)
    d1 = slot_of[d1]
    s2c = slot_of[s2c]

    # ---- L1 packing: (core, window, lo/hi) buckets
    core1 = d1 // M1
    win1 = (d1 % M1) // WIN
    slot1 = d1 % WIN
    hi1 = (e1s >= HIB).astype(np.int64)
    key1 = (core1 * NW1 + win1) * 2 + hi1
    nk1 = NCORES * NW1 * 2
    cnt1 = np.bincount(key1, minlength=nk1).reshape(NCORES, NW1, 2)
    K1LO = int(-(-cnt1[:, :, 0].max() // 128))
    K1HI = int(-(-cnt1[:, :, 1].max() // 128))
    K1 = K1LO + K1HI

    # secondary sort by src id: ascending gather addresses within a bucket
    order = np.lexsort((e1s, key1))
    ks = key1[order]
    starts = np.searchsorted(ks, np.arange(nk1))
    rank = np.arange(len(ks)) - starts[ks]
    base = np.where(ks % 2 == 0, 0, K1LO * 128)
    pos = rank + base  # slot within window stream

    idx1 = np.zeros((NCORES, NW1, K1 * 128), np.int32)
    w1s = np.zeros((NCORES, NW1, K1 * 128), np.float32)
    d1s = np.zeros((NCORES, NW1, K1 * 128), np.float32)
    oc = core1[order]
    ow = win1[order]
    sv = e1s[order] - np.where(ks % 2 == 0, 0, HIB)
    idx1[oc, ow, pos] = sv
    w1s[oc, ow, pos] = e1w[order]
    d1s[oc, ow, pos] = slot1[order]

    # ---- L2 packing: (owner-core-by-src, window) buckets
    core2 = s2c // M1
    loc2 = s2c % M1
    win2 = d2 // WIN
    slot2 = d2 % WIN
    key2 = core2 * NW2 + win2
    nk2 = NCORES * NW2
    cnt2 = np.bincount(key2, minlength=nk2).reshape(NCORES, NW2)
    K2 = int(-(-cnt2.max() // 128))

    order = np.lexsort((loc2, key2))
    ks = key2[order]
    starts = np.searchsorted(ks, np.arange(nk2))
    rank = np.arange(len(ks)) - starts[ks]
    idx2 = np.zeros((NCORES, NW2, K2 * 128), np.int32)
    w2s = np.zeros((NCORES, NW2, K2 * 128), np.float32)
    d2s = np.zeros((NCORES, NW2, K2 * 128), np.float32)
    oc = core2[order]
    ow = win2[order]
    idx2[oc, ow, rank] = loc2[order]
    w2s[oc, ow, rank] = e2w[order]
    d2s[oc, ow, rank] = slot2[order]

    # ---- meta layout (shared columns for all cores; stream parts per-core)
    iota = np.broadcast_to(np.arange(WIN, dtype=np.float32), (128, WIN))
    b1b = np.broadcast_to(b1.astype(np.float32), (128, 128))
    b2b = np.broadcast_to(b2.astype(np.float32), (128, 40))
    ident = np.eye(128, dtype=np.float32)
    W1a16 = W1[0:128, :].astype(NPBF16)
    W1b16 = W1[128:256, :].astype(NPBF16)
    W2c16 = W2.astype(NPBF16)  # [128, 40]
    meta16 = np.ascontiguousarray(
        np.concatenate([W1a16, W1b16, W2c16], axis=1))

    x16 = np.ascontiguousarray(x.astype(NPBF16))

    in_maps = []
    for c in range(NCORES):
        m = [iota,
             _chunk_major(w1s[c], K1), _chunk_major(d1s[c], K1),
             -_chunk_major(w1s[c], K1), -_chunk_major(d1s[c], K1),
             _chunk_major(w2s[c], K2), _chunk_major(d2s[c], K2),
             ident, b1b, b2b]
        meta = np.ascontiguousarray(np.concatenate(m, axis=1))
        in_maps.append({
            "x": x16,
            "idx1lo": _pack_idx16(
                idx1[c, :, :K1LO * 128].reshape(-1)),
            "idx1hi": _pack_idx16(
                idx1[c, :, K1LO * 128:].reshape(-1)),
            "idx2": _pack_idx16(idx2[c].reshape(-1)),
            "meta": meta,
            "meta16": meta16,
        })
    return in_maps, (K1LO, K1HI, K2), idx_u, idx_inv, n2


# --------------------------------------------------------------- the program
def _meta_cols(K1, K2):
    """Column offsets within meta (f32)."""
    o = {}
    cur = 0
    for name, width in [("iota", WIN),
                        ("w1", NW1 * K1), ("d1", NW1 * K1),
                        ("nw1", NW1 * K1), ("nd1", NW1 * K1),
                        ("w2", NW2 * K2), ("d2", NW2 * K2),
                        ("ident", 128), ("b1b", 128), ("b2b", 40)]:
        o[name] = cur
        cur += width
    o["total"] = cur
    return o


M16_COLS = {"W1a": 0, "W1b": 128, "W2": 256, "total": 296}


def _build(n_nodes, K1LO, K1HI, K2, reps=1):
    K1 = K1LO + K1HI
    MC = _meta_cols(K1, K2)
    nc = bacc.Bacc("TRN2", target_bir_lowering=False, debug=False,
                   num_devices=NCORES)

    x_t = nc.dram_tensor("x", [n_nodes, 256], BF16, kind="ExternalInput")
    i1lo_t = nc.dram_tensor("idx1lo", [128, NW1 * K1LO * 8], I16,
                            kind="ExternalInput")
    i1hi_t = nc.dram_tensor("idx1hi", [128, NW1 * K1HI * 8], I16,
                            kind="ExternalInput")
    i2_t = nc.dram_tensor("idx2", [128, NW2 * K2 * 8], I16,
                          kind="ExternalInput")
    meta_t = nc.dram_tensor("meta", [128, MC["total"]], F32,
                            kind="ExternalInput")
    meta16_t = nc.dram_tensor("meta16", [128, M16_COLS["total"]], BF16,
                              kind="ExternalInput")
    out_t = nc.dram_tensor("out", [2 * 128, 40], F32, kind="ExternalOutput")
    h1_d = nc.dram_tensor("h1_d", [M1, 128], BF16)
    # dst-major partial sums: [block(=2*win+half), slot, hid]
    ar_in = nc.dram_tensor("ar_in", [2 * NW2, 128, 128], F32)
    rs_out = nc.dram_tensor("rs_out", [2 * NW2 // NCORES, 128, 128], F32)

    import contextlib
    ctx = contextlib.ExitStack()
    sb = lambda name, shape, dt: ctx.enter_context(
        nc.sbuf_tensor(name, shape, dt))
    ps = lambda name, shape: ctx.enter_context(
        nc.psum_tensor(name, shape, F32))
    sem = lambda name: ctx.enter_context(nc.semaphore(name))

    i1lo = sb("i1lo", [128, NW1 * K1LO * 8], I16)
    i1hi = sb("i1hi", [128, NW1 * K1HI * 8], I16)
    i2 = sb("i2", [128, NW2 * K2 * 8], I16)
    meta = sb("meta_sb", [128, MC["total"]], F32)
    meta16 = sb("meta16_sb", [128, M16_COLS["total"]], BF16)
    H1 = sb("H1", [128, 2, K1, 256], BF16)
    Srng = sb("Srng", [128, 2 * K1, WIN], BF16)
    H2 = sb("H2", [128, 2, K2, 128], BF16)
    spT = sb("spT", [128, 2, 2, 256], BF16)
    h1sb = sb("h1sb", [128, 2, 2, 128], BF16)
    l2pT = sb("l2pT", [128, 2 * NW2, 128], F32)
    arsb = sb("arsb", [128, 2, 128], F32)
    arsT = sb("arsT", [128, 2, 128], BF16)
    lssb = sb("lssb", [128, 2, 40], F32)
    escr = sb("escr", [128, 2, 40], F32)
    red = sb("red", [128, 64], F32)
    uscr = sb("uscr", [128, 2, 4, WIN], F32)

    # one full 2KB bank per tensor: avoids cross-tensor same-bank
    # PE-write/DVE-read hazards
    psA = [ps("psA0", [128, 512]), ps("psA1", [128, 512])]
    psB = [ps("psB0", [128, 512]), ps("psB1", [128, 512])]
    psH = [ps("psH0", [128, 512]), ps("psH1", [128, 512])]

    hw_s = sem("hw_s")
    g_r = [sem(f"g_r{i}") for i in range(4)]
    h1_r = [sem(f"h1_r{i}") for i in range(2)]
    out_r = [sem(f"out_r{i}") for i in range(2)]
    s_s = sem("s_s")
    pe_s = sem("pe_s")
    v_s = sem("v_s")
    a_s = sem("a_s")
    c_s = sem("c_s")

    # ---- schedule builder: append closures per engine with exact sem counts
    SP, PL, PE, DVE, ACT = "sp", "pl", "pe", "dve", "act"
    sched = {e: [] for e in (SP, PL, PE, DVE, ACT)}
    cnt = dict(hw=0, s=0, pe=0, v=0, a=0, c=0,
               g0=0, g1=0, g2=0, g3=0, h0=0, h1=0, o0=0, o1=0)
    semmap = dict(hw=hw_s, s=s_s, pe=pe_s, v=v_s, a=a_s, c=c_s,
                  g0=g_r[0], g1=g_r[1], g2=g_r[2], g3=g_r[3],
                  h0=h1_r[0], h1=h1_r[1], o0=out_r[0], o1=out_r[1])

    def wait(e, semname, val):
        if val <= 0:
            return
        s_ = semmap[semname]
        sched[e].append(lambda eng: eng.wait_ge(s_, val))

    def bump(semname, n=1):
        cnt[semname] += n
        return cnt[semname]

    def op(e, fn, semname=None, n=1):
        """fn(eng) must return the instruction; attach then_inc."""
        if semname is None:
            sched[e].append(fn)
            return None
        s_ = semmap[semname]
        v = bump(semname, n)
        sched[e].append(lambda eng: fn(eng).then_inc(s_, n))
        return v

    n_dve1 = K1 - int(round(K1 * ACT_FRAC))  # chunks/window on DVE

    # --- preamble: index buffers first (gathers depend only on these)
    op(SP, lambda e: e.dma_start(out=i1lo[:], in_=i1lo_t.ap()), "hw", 16)
    hw_idx1 = op(SP, lambda e: e.dma_start(out=i1hi[:], in_=i1hi_t.ap()),
                 "hw", 16)
    hw_idx2 = op(SP, lambda e: e.dma_start(out=i2[:], in_=i2_t.ap()),
                 "hw", 16)
    op(SP, lambda e: e.dma_start(out=meta[:], in_=meta_t.ap()), "hw", 16)
    hw_pre = op(SP, lambda e: e.dma_start(out=meta16[:], in_=meta16_t.ap()),
                "hw", 16)
    sched[PL].append(lambda eng: eng.load_library(library_config.mlp))
    regs = {}
    GMAX = 8  # max chunks per dma_gather (>1024 idxs per instr breaks)

    def pieces(K):
        out = []
        c0 = 0
        while c0 < K:
            c1 = min(c0 + GMAX, K)
            out.append((c0, c1))
            c0 = c1
        return out

    def plreg(val):
        def fn(eng):
            regs[val] = eng.to_reg(val)
        sched[PL].append(fn)
    _sizes = set()
    for K in (K1LO, K1HI, K2):
        for c0, c1 in pieces(K):
            _sizes.add((c1 - c0) * 128)
    for v in sorted(_sizes):
        plreg(v)
    wait(PL, "hw", hw_idx1)
    wait(DVE, "hw", hw_pre)
    wait(ACT, "hw", hw_pre)
    wait(PE, "hw", hw_pre)

    def mcol(name, k=0, width=1):
        c0 = MC[name] + k * width
        return meta[:, c0:c0 + width]

    def m16col(name, width):
        c0 = M16_COLS[name]
        return meta16[:, c0:c0 + width]

    def s_build(e, slot, ccol, layer):
        """one S chunk build on DVE. ccol = stream column index."""
        w_ap = mcol("w1" if layer == 1 else "w2", ccol)
        d_ap = mcol("d1" if layer == 1 else "d2", ccol)
        Sap = Srng[:, slot, :]
        op(DVE, lambda eng: eng.tensor_scalar(
            Sap, meta[:, 0:WIN], d_ap, w_ap, AL.is_equal, AL.mult),
            "s")

    act_grp = {"i": 0, "prev_relu": {}}

    def s_build_act_group(slots_ccols):
        """<=4 S chunks on ACT: Abs batch -> sem -> Relu batch -> sem.
        ACT has no auto-drain, so RAW/WAR need explicit sem round-trips."""
        gi = act_grp["i"]
        bank = gi % 2
        # WAR: this group's Abs overwrites the bank read by Relus of group
        # gi-2; those Relus inc a_s, so a value-based wait orders us.
        if gi - 2 in act_grp["prev_relu"]:
            wait(ACT, "a", act_grp["prev_relu"][gi - 2])
        last = None
        for i, (slot, ccol) in enumerate(slots_ccols):
            nd_ap = mcol("nd1", ccol)
            last = op(ACT, lambda eng, i=i, nd_ap=nd_ap: eng.activation(
                uscr[:, bank, i, :], meta[:, 0:WIN], AF.Abs, bias=nd_ap),
                "a")
        wait(ACT, "a", last)
        rl = None
        for i, (slot, ccol) in enumerate(slots_ccols):
            w_ap = mcol("w1", ccol)
            nw_ap = mcol("nw1", ccol)
            rl = op(ACT, lambda eng, i=i, slot=slot, w_ap=w_ap, nw_ap=nw_ap:
                    eng.activation(
                Srng[:, slot, :], uscr[:, bank, i, :], AF.Relu,
                bias=w_ap, scale=nw_ap), "a")
        act_grp["prev_relu"][gi] = rl
        act_grp["i"] = gi + 1
        return rl

    def rep_barrier():
        # full cross-engine barrier between repetitions
        engines = (SP, PL, PE, DVE, ACT)
        names = ("pe", "v", "a", "s", "c", "hw",
                 "g0", "g1", "g2", "g3", "h0", "h1", "o0", "o1")
        for e in engines:
            for nm in names:
                wait(e, nm, cnt[nm])

    def emit_pipeline():
        pe_chunks = {}
        pe_w1 = {}
        copyv = {}
        biasv = {}
        reluv = {}
        h1hw = {}
        gv1 = {}
        sv1 = {}

        # --- L1 pipeline
        for w in range(NW1 + 1):
            if w < NW1:
                r = w % 2
                if w >= 2:
                    wait(PL, "pe", pe_chunks[w - 2])
                for c0, c1 in pieces(K1LO):
                    n_ = (c1 - c0) * 128
                    op(PL, lambda e, r=r, w=w, c0=c0, c1=c1, n_=n_:
                       e.dma_gather(
                        H1[:, r, c0:c1, :], x_t.ap(),
                        i1lo[:, (w * K1LO + c0) * 8:(w * K1LO + c1) * 8],
                        n_, regs[n_], 256), f"g{w % 4}", 16)
                for c0, c1 in pieces(K1HI):
                    n_ = (c1 - c0) * 128
                    gv1[w] = op(PL, lambda e, r=r, w=w, c0=c0, c1=c1, n_=n_:
                                e.dma_gather(
                        H1[:, r, K1LO + c0:K1LO + c1, :], x_t.ap()[HIB:, :],
                        i1hi[:, (w * K1HI + c0) * 8:(w * K1HI + c1) * 8],
                        n_, regs[n_], 256), f"g{w % 4}", 16)
                if w >= 2:
                    wait(DVE, "pe", pe_chunks[w - 2])
                    wait(ACT, "pe", pe_chunks[w - 2])
                for c in range(n_dve1):
                    s_build(DVE, r * K1 + c, w * K1 + c, 1)
                acts = [(r * K1 + c, w * K1 + c)
                        for c in range(n_dve1, K1)]
                av1 = None
                for j in range(0, len(acts), 4):
                    av1 = s_build_act_group(acts[j:j + 4])
                sv1[w] = cnt["s"]
                wait(PE, f"g{w % 4}", gv1[w])
                wait(PE, "s", sv1[w])
                if av1 is not None:
                    wait(PE, "a", av1)
                if w >= 2:
                    wait(PE, "v", copyv[w - 2])
                for c in range(K1):
                    st, sp_ = (c == 0), (c == K1 - 1)
                    op(PE, lambda e, r=r, c=c, st=st, sp_=sp_: e.matmul(
                        psA[r][:, 0:256], H1[:, r, c, 0:128],
                        Srng[:, r * K1 + c, :],
                        start=st, stop=sp_, skip_group_check=True))
                    last = op(PE, lambda e, r=r, c=c, st=st, sp_=sp_:
                              e.matmul(
                        psB[r][:, 0:256], H1[:, r, c, 128:256],
                        Srng[:, r * K1 + c, :],
                        start=st, stop=sp_, skip_group_check=True),
                        "pe" if sp_ else None)
                pe_chunks[w] = last
            if w >= 1:
                wa = w - 1
                ra = wa % 2
                wait(DVE, "pe", pe_chunks[wa])
                op(DVE, lambda e, ra=ra: e.tensor_copy(
                    spT[:, ra, 0, :], psA[ra][:, 0:256]), "v")
                copyv[wa] = op(DVE, lambda e, ra=ra: e.tensor_copy(
                    spT[:, ra, 1, :], psB[ra][:, 0:256]), "v")
                wait(PE, "v", copyv[wa])
                if wa >= 2:
                    wait(PE, "a", reluv[wa - 2])
                for dh in range(2):
                    for k in range(2):
                        pe_w1[wa] = op(PE, lambda e, ra=ra, dh=dh, k=k:
                                       e.matmul(
                            psH[ra][:, dh * 128:(dh + 1) * 128],
                            spT[:, ra, k, dh * 128:(dh + 1) * 128],
                            m16col("W1a" if k == 0 else "W1b", 128),
                            start=(k == 0), stop=(k == 1),
                            skip_group_check=True),
                            "pe" if (dh == 1 and k == 1) else None)
                wait(DVE, "pe", pe_w1[wa])
                for dh in range(2):
                    biasv[wa] = op(DVE, lambda e, ra=ra, dh=dh:
                                   e.tensor_tensor(
                        psH[ra][:, dh * 128:(dh + 1) * 128],
                        psH[ra][:, dh * 128:(dh + 1) * 128],
                        mcol("b1b", 0, 128), AL.add), "v")
                wait(ACT, "v", biasv[wa])
                if wa >= 2:
                    wait(ACT, f"h{wa % 2}", h1hw[wa - 2])
                for dh in range(2):
                    reluv[wa] = op(ACT, lambda e, ra=ra, dh=dh: e.activation(
                        h1sb[:, ra, dh, :],
                        psH[ra][:, dh * 128:(dh + 1) * 128],
                        AF.Relu), "a")
                wait(SP, "a", reluv[wa])
                for dh in range(2):
                    h1hw[wa] = op(SP, lambda e, ra=ra, wa=wa, dh=dh:
                                  e.dma_start(
                        out=h1_d.ap()[wa * 256 + dh * 128:
                                      wa * 256 + (dh + 1) * 128, :],
                        in_=h1sb[:, ra, dh, :]), f"h{wa % 2}", 16)

        # --- L2 pipeline (flipped orientation: psum tiles are [dst, hid])
        wait(PL, "h0", cnt["h0"])
        wait(PL, "h1", cnt["h1"])
        wait(PL, "hw", hw_idx2)
        wait(DVE, "pe", pe_chunks[NW1 - 1])
        wait(ACT, "pe", pe_chunks[NW1 - 1])
        pe_l2 = {}
        gv2 = {}
        sv2 = {}
        cl2 = {}
        for v2 in range(NW2 + 1):
            if v2 < NW2:
                r = v2 % 2
                if v2 >= 2:
                    wait(PL, "pe", pe_l2[v2 - 2])
                for c0, c1 in pieces(K2):
                    n_ = (c1 - c0) * 128
                    gv2[v2] = op(PL, lambda e, r=r, v2=v2, c0=c0, c1=c1,
                                 n_=n_: e.dma_gather(
                        H2[:, r, c0:c1, :], h1_d.ap(),
                        i2[:, (v2 * K2 + c0) * 8:(v2 * K2 + c1) * 8],
                        n_, regs[n_], 128), f"g{v2 % 4}", 16)
                if v2 >= 2:
                    wait(DVE, "pe", pe_l2[v2 - 2])
                for c in range(K2):
                    s_build(DVE, r * K1 + c, v2 * K2 + c, 2)
                sv2[v2] = cnt["s"]
                wait(PE, f"g{v2 % 4}", gv2[v2])
                wait(PE, "s", sv2[v2])
                wait(PE, "v", copyv[NW1 - 2 + r] if v2 < 2 else cl2[v2 - 2])
                for half in range(2):
                    for c in range(K2):
                        st, sp_ = (c == 0), (c == K2 - 1)
                        pe_l2[v2] = op(
                            PE, lambda e, r=r, c=c, half=half, st=st,
                            sp_=sp_: e.matmul(
                                psA[r][:, half * 128:(half + 1) * 128],
                                Srng[:, r * K1 + c,
                                     half * 128:(half + 1) * 128],
                                H2[:, r, c, :],
                                start=st, stop=sp_, skip_group_check=True),
                            "pe" if (half == 1 and sp_) else None)
            if v2 >= 1:
                va = v2 - 1
                ra = va % 2
                wait(DVE, "pe", pe_l2[va])
                for half in range(2):
                    cl2[va] = op(DVE, lambda e, ra=ra, va=va, half=half:
                                 e.tensor_copy(
                        l2pT[:, 2 * va + half, :],
                        psA[ra][:, half * 128:(half + 1) * 128]), "v")

        # --- ReduceScatter along dst: core c receives blocks 2c, 2c+1
        wait(SP, "v", cl2[NW2 - 1])
        hw_l2p = op(SP, lambda e: e.dma_start(
            out=ar_in.ap().transpose([1, 0, 2]), in_=l2pT[:, :, :]),
            "hw", 16)
        wait(PL, "hw", hw_l2p)
        op(PL, lambda e: e.collective_compute(
            "ReduceScatter", AL.add, ins=[ar_in.ap()], outs=[rs_out.ap()],
            replica_groups=[list(range(NCORES))]), "c")
        wait(SP, "c", cnt["c"])
        hw_ar = op(SP, lambda e: e.dma_start(
            out=arsb[:, :, :], in_=rs_out.ap().transpose([1, 0, 2])),
            "hw", 16)

        # --- final: W2 matmul + bias + log_softmax on this core's 256 rows.
        # Exp/Ln are batched across the two blocks so the activation table
        # only has to swap twice per iteration.
        wait(PE, "hw", hw_ar)
        wait(PE, "v", copyv[NW1 - 1])
        wait(PE, "a", reluv[NW1 - 1])
        tp = {}
        for j in range(2):
            tp[j] = op(PE, lambda e, j=j: e.transpose(
                psH[j][:, 0:128], arsb[:, j, :], mcol("ident", 0, 128)),
                "pe")
        wait(DVE, "pe", tp[1])
        ct = {}
        for j in range(2):
            ct[j] = op(DVE, lambda e, j=j: e.tensor_copy(
                arsT[:, j, :], psH[j][:, 0:128]), "v")
        wait(PE, "v", ct[1])
        wm = {}
        for j in range(2):
            wm[j] = op(PE, lambda e, j=j: e.matmul(
                psB[j][:, 0:40], arsT[:, j, :], m16col("W2", 40),
                start=True, stop=True, skip_group_check=True), "pe")
        wait(DVE, "pe", wm[1])
        nmv = {}
        for j in range(2):
            op(DVE, lambda e, j=j: e.tensor_tensor(
                psB[j][:, 0:40], psB[j][:, 0:40], mcol("b2b", 0, 40),
                AL.add), "v")
            nmv[j] = op(DVE, lambda e, j=j: e.tensor_reduce(
                red[:, j * 8:j * 8 + 1], psB[j][:, 0:40], AX.X, AL.max,
                negate=True), "v")
        wait(ACT, "v", nmv[1])
        if reps > 1:
            wait(ACT, "o0", cnt["o0"])  # lssb WAR across reps
        ev = {}
        for j in range(2):
            ev[j] = op(ACT, lambda e, j=j: e.activation(
                escr[:, j, :], psB[j][:, 0:40], AF.Exp,
                bias=red[:, j * 8:j * 8 + 1],
                accum_out=red[:, j * 8 + 1:j * 8 + 2]), "a")
        wait(DVE, "a", ev[1])
        rv = {}
        for j in range(2):
            rv[j] = op(DVE, lambda e, j=j: e.reciprocal(
                red[:, j * 8 + 2:j * 8 + 3],
                red[:, j * 8 + 1:j * 8 + 2]), "v")
        wait(ACT, "v", rv[1])
        lv = {}
        for j in range(2):
            lv[j] = op(ACT, lambda e, j=j: e.activation(
                red[:, j * 8 + 3:j * 8 + 4],
                red[:, j * 8 + 2:j * 8 + 3], AF.Ln), "a")
        wait(DVE, "a", lv[1])
        mv = {}
        for j in range(2):
            mv[j] = op(DVE, lambda e, j=j: e.tensor_tensor(
                red[:, j * 8 + 4:j * 8 + 5], red[:, j * 8:j * 8 + 1],
                red[:, j * 8 + 3:j * 8 + 4], AL.add), "v")
        wait(ACT, "v", mv[1])
        fin = {}
        for j in range(2):
            fin[j] = op(ACT, lambda e, j=j: e.activation(
                lssb[:, j, :], psB[j][:, 0:40], AF.Identity,
                bias=red[:, j * 8 + 4:j * 8 + 5]), "a")
        wait(SP, "a", fin[1])
        for j in range(2):
            op(SP, lambda e, j=j: e.dma_start(
                out=out_t.ap()[j * 128:(j + 1) * 128, :],
                in_=lssb[:, j, :]), "o0", 16)

    emit_pipeline()
    for _ in range(reps - 1):
        rep_barrier()
        emit_pipeline()

    # ---- emit engine bodies
    with nc.Block() as block:
        @block.sync
        def _(eng):
            for fn in sched[SP]:
                fn(eng)

        @block.gpsimd
        def _(eng):
            for fn in sched[PL]:
                fn(eng)

        @block.tensor
        def _(eng):
            for fn in sched[PE]:
                fn(eng)

        @block.vector
        def _(eng):
            for fn in sched[DVE]:
                fn(eng)

        @block.scalar
        def _(eng):
            for fn in sched[ACT]:
                fn(eng)

    ctx.close()
    nc.compile()
    return nc


def get_program(n_nodes, K1LO, K1HI, K2, reps=1):
    key = (n_nodes, K1LO, K1HI, K2, reps)
    if key not in _PROG_CACHE:
        _PROG_CACHE[key] = _build(n_nodes, K1LO, K1HI, K2, reps)
    return _PROG_CACHE[key]


# ------------------------------------------------------------------- kernel
def kernel(x, W1, b1, W2, b2, edge_weight, src, dst, idx, _trace=False):
    x = np.asarray(x, np.float32)
    in_maps, (K1LO, K1HI, K2), idx_u, idx_inv, n2 = _prep(
        x, np.asarray(W1, np.float32), np.asarray(b1, np.float32),
        np.asarray(W2, np.float32), np.asarray(b2, np.float32),
        np.asarray(edge_weight, np.float32),
        np.asarray(src, np.int32), np.asarray(dst, np.int32),
        np.asarray(idx, np.int32))
    nc = get_program(x.shape[0], K1LO, K1HI, K2)
    res = run_bass_kernel_spmd(nc, in_maps, core_ids=list(range(NCORES)),
                               trace=_trace)
    # core c's [256, 40] output covers dst slots [256c, 256(c+1))
    full = np.concatenate([res.results[c]["out"] for c in range(NCORES)],
                          axis=0)
    out = full[idx_inv].astype(np.float32)
    if _trace:
        kernel.last_exec_time_ns = res.exec_time_ns
    return out


kernel.last_exec_time_ns = None
